# revision 1
# baseline (speedup 1.0000x reference)
"""RBF kernel matrix K[i,j] = exp(-||x_i - y_j||^2) on 8 trn2 NeuronCores.

Strategy (hardcoded for x:[8192,256] f32, y:[8192,256] f32):
  - Shard rows of x across the 8 cores (1024 rows each); replicate y.
  - Use the expansion -dist2 = (x . 2y) - x2_i - y2_j, computed as one
    augmented GEMM on the tensor engine:
        PSUM[i,j] = sum_d xT[d,i] * yT2[d,j]  +  xe[:,i] . ye[:,j]
    where xe = [x2_i; 1], ye = [-1; -y2_j] (a K=2 matmul adds the rank-2
    bias term), then exp() on the scalar engine straight out of PSUM.
  - Host precomputes the bf16 transposed operands and the row norms; the
    device kernel is pure GEMM + exp + writeout (output DMA bound).
"""

import numpy as np
import ml_dtypes

N = 8192
M = 8192
D = 256
NCORES = 8
RPC = N // NCORES  # rows of x per core: 1024

_cached = {}


def _build():
    import concourse.tile as tile
    import concourse.mybir as mybir
    from concourse import bacc

    f32 = mybir.dt.float32
    bf16 = mybir.dt.bfloat16
    fp8 = mybir.dt.float8e4

    nc = bacc.Bacc("TRN2", target_bir_lowering=False)

    xT = nc.dram_tensor("xT", [D, RPC], fp8, kind="ExternalInput")
    xe = nc.dram_tensor("xe", [2, RPC], bf16, kind="ExternalInput")
    yT = nc.dram_tensor("yT", [D, M], fp8, kind="ExternalInput")
    ye = nc.dram_tensor("ye", [2, M], bf16, kind="ExternalInput")
    out = nc.dram_tensor("out", [RPC, M], f32, kind="ExternalOutput")

    JT = 512          # matmul free dim (one PSUM bank)
    JG = 2048         # PSUM group: 4 banks, one exp + one store per group
    NIG = RPC // 128  # 8 i-blocks
    NG = M // JG      # 4 j-groups
    KC = D // 128     # 2 contraction chunks

    xT_ap = xT[:].rearrange("(c p) f -> p c f", p=128)
    yT_ap = yT[:].rearrange("(c p) f -> p c f", p=128)
    out_ap = out[:].rearrange("(g p) f -> g p f", p=128)

    with tile.TileContext(nc) as tc:
        with (
            tc.tile_pool(name="consts", bufs=1) as consts,
            tc.tile_pool(name="outsb", bufs=3) as outsb,
            tc.tile_pool(name="psum", bufs=2, space="PSUM") as psum,
        ):
            # Inputs go through the Scalar HWDGE ring so the Sync ring is
            # dedicated to output stores (HWDGE is FIFO per issuing engine).
            xT_sb = consts.tile([128, KC, RPC], fp8)
            nc.scalar.dma_start(xT_sb[:], xT_ap)
            xe_sb = consts.tile([2, RPC], bf16)
            nc.scalar.dma_start(xe_sb[:], xe[:])
            ye_sb = consts.tile([2, M], bf16)
            nc.scalar.dma_start(ye_sb[:], ye[:])
            yT_sb = consts.tile([128, KC, M], fp8)
            for g in range(NG):
                nc.scalar.dma_start(
                    yT_sb[:, :, g * JG:(g + 1) * JG], yT_ap[:, :, g * JG:(g + 1) * JG]
                )

            SG = 2  # psum groups staged per output DMA (2 MiB stores)
            for ig in range(NIG):
                i0 = ig * 128
                for g0 in range(0, NG, SG):
                    ot = outsb.tile([128, SG * JG], f32)
                    for g in range(g0, g0 + SG):
                        j0 = g * JG
                        pt = psum.tile([128, JG], f32)
                        for c in range(KC):
                            for jj in range(JG // JT):
                                nc.tensor.matmul(
                                    pt[:, jj * JT:(jj + 1) * JT],
                                    lhsT=xT_sb[:, c, i0:i0 + 128],
                                    rhs=yT_sb[:, c, j0 + jj * JT:j0 + (jj + 1) * JT],
                                    start=(c == 0),
                                    stop=False,
                                )
                        for jj in range(JG // JT):
                            nc.tensor.matmul(
                                pt[:, jj * JT:(jj + 1) * JT],
                                lhsT=xe_sb[:, i0:i0 + 128],
                                rhs=ye_sb[:, j0 + jj * JT:j0 + (jj + 1) * JT],
                                start=False,
                                stop=True,
                            )
                        nc.scalar.activation(
                            ot[:, (g - g0) * JG:(g - g0 + 1) * JG], pt[:],
                            mybir.ActivationFunctionType.Exp,
                        )
                    if ig == 0 and g0 == 0:
                        # split the first store so the output stream starts
                        # as soon as the first exp tile exists
                        nc.sync.dma_start(out_ap[ig, :, :JG], ot[:, :JG])
                        nc.sync.dma_start(out_ap[ig, :, JG:SG * JG], ot[:, JG:])
                    else:
                        nc.sync.dma_start(
                            out_ap[ig, :, g0 * JG:(g0 + SG) * JG],
                            ot[:],
                        )

    nc.compile()
    return nc


def _prep_inputs(x: np.ndarray, y: np.ndarray):
    bf16 = ml_dtypes.bfloat16
    fp8 = ml_dtypes.float8_e4m3
    x = np.asarray(x, dtype=np.float32)
    y = np.asarray(y, dtype=np.float32)
    x2 = np.sum(x * x, axis=1)  # [N]
    y2 = np.sum(y * y, axis=1)  # [M]

    yT = np.ascontiguousarray(np.transpose(2.0 * y)).astype(fp8)  # [D, M]
    ye = np.empty((2, M), dtype=bf16)
    ye[0] = bf16(-1.0)
    ye[1] = (-y2).astype(bf16)

    in_maps = []
    for c in range(NCORES):
        sl = slice(c * RPC, (c + 1) * RPC)
        xT_c = np.ascontiguousarray(np.transpose(x[sl])).astype(fp8)  # [D, RPC]
        xe_c = np.empty((2, RPC), dtype=bf16)
        xe_c[0] = x2[sl].astype(bf16)
        xe_c[1] = bf16(1.0)
        in_maps.append({"xT": xT_c, "xe": xe_c, "yT": yT, "ye": ye})
    return in_maps


def kernel(x: np.ndarray, y: np.ndarray, _trace: bool = False):
    from concourse.bass_utils import run_bass_kernel_spmd

    if "nc" not in _cached:
        _cached["nc"] = _build()
    nc = _cached["nc"]

    in_maps = _prep_inputs(x, y)
    res = run_bass_kernel_spmd(
        nc, in_maps, core_ids=list(range(NCORES)), trace=_trace
    )
    outp = np.concatenate([res.results[c]["out"] for c in range(NCORES)], axis=0)
    if _trace:
        _cached["last_result"] = res
    return outp

